# revision 38
# baseline (speedup 1.0000x reference)
"""Trainium2 Bass kernel for nn_Attention_9242769622327.

Math: the reference computes
    qkv = x @ W1.T ; q,k,v = split(qkv)
    score = softmax(k^T v / 4, axis=-1)            # rows sum to 1
    attn  = softmax(einsum('bhnk,bhkc->bhnk', q/4, score), axis=-1)
          = softmax(q/4)                           # sum_c score == 1, k/v dead
    out   = attn @ W2.T
so only the q-projection (first E rows of W1), a per-head (64-wide) softmax,
and the output projection are needed.

Distribution: pure data-parallel over the 32768 = B*S rows; each of the 8
cores handles 4096 rows with the full weights. No collectives.

Precision: EVERY matmul runs in fp8-e4m3 DoubleRow (2 fp8 MACs per PE cell
per cycle -> half the matmul instructions of bf16, and no PE dtype-mode
switches anywhere -- v2 of this kernel measured fp16 matmuls embedded in a
DR stream at 400-570ns instead of 213ns).  Two fp8 tricks keep the error
inside the 2% gate:

1. mm2 on centered attention.  attn rows sum to exactly 1 per 64-wide head,
   so  out[j,m] = K[j] + sum_n W2T[n,j] * d[n,m]  with  d = attn - 1/64 and
   K[j] = sum_n W2T[n,j]/64 a CONSTANT added on the host in fp32.  The
   deviation d is ~4x smaller than attn, so fp8 noise on it (and on an fp8
   W2) lands at ~0.6% each on the output instead of ~2.5%.

2. hi/lo fp8 split of the softmax reciprocal.  The per-head normalizer
   rcp = 64/s must reach all 64 partitions of its head, which only the PE
   (broadcast matmul) can do cheaply; fp8 DR needs fp8 operands but fp8
   alone would put ~2% on the output.  So rcp is split as
   hi8 = fp8(rcp), lo8 = fp8(rcp - hi8)  (two tiny [16,MS] engine ops) and
   packed into one DR pair-operand; the 0/1 selector sums the pair back:
   rb = hi8 + lo8 ~= rcp to ~0.1%.  (A DMA-ring broadcast was tried instead
   and is memory-unfriendly: it adds 8.4MB/core to a HW-DGE ring that
   saturates at ~105GB/s.)

Scales: W1q and W2T are host-prescaled by 32 (entries std 1/32 -> 1), the
head-sum selector holds 1/64 so the head-sum PSUM is s/64 and its
reciprocal 64/s; then at = u*(64/s) = 64*attn, d8 = fp8(at - 1) = 64*d,
and the host divides the gathered output by 32*64 = 2048 before adding K.

On-chip layout is fully transposed (features on partitions, rows on the free
dim) so no on-chip transposes are needed anywhere:
    qT[n,m]  = sum_k W1qT[k,n] * xT[k,m]          (PE, fp8 DR, K=256/MM)
    u        = exp(qT/128)                        (ACT, PSUM->SBUF fp16)
    u8       = fp8(u)                             (DVE, one whole-strip copy)
    s[g,m]   = sum_{n in head g} u8[n,m] / 64     (PE fp8-DR w/ 1/64 selector)
    rcp      = 1/s = 64/head-sum                  (DVE reciprocal_approx_fast)
    hi8/lo8  = fp8 split of rcp                   (ACT copy + DVE sub, [16,MS])
    rb[n,m]  = rcp[head(n),m]                     (PE fp8-DR selector matmul)
    at       = u * rb                             (DVE, per chunk from PSUM)
    d8       = fp8(at - 1)                        (DVE, one whole-strip add)
    outT[j,m]= sum_n (32*W2T)[n,j] * d8[n,m]      (PE, fp8 DR)

Software pipeline, two stripes deep (the rb->at->d8 chain of stripe ms-1
hides under mm2 of stripe ms-2); per iteration the PE runs
  [32 mm1(ms)] [8 rb(ms-1) | 32 mm2(ms-2), interleaved 2 rb : 1 j-block]
  [4 hs(ms)]  = 76 DR MMs ~= 16.2us at the 213ns/MM N=512 stream floor.
The 2:1 interleave paces the 2-bank rb PSUM rotation against the DVE
at-mul drains with no PE stall.  Engine FIFOs:
  ACT: [8 exp(ms)] [8 o-drains(ms-2)] [hi8(ms)]
  DVE: [8 at(ms-1)] [u8m(ms)] [d8m(ms-1)] [recip(ms)] [lo8(ms)]
Whole-strip [128, 4096] DVE ops amortize the ~320ns per-op engine overhead.
PSUM banks: 3 q + 2 rb + 2 out + 1 head-sum = 8.
w2/sel_rb DMAs are deferred behind the stripe-0/1 x + w1 loads; 8 throwaway
matmuls on memset scratch warm the PE HAM clock gate during that window.
x/w1/w2 are host-packed so every DMA tile is one contiguous DRAM block.
"""

import sys

sys.path.insert(0, "/opt/trn_rl_repo")

import numpy as np
import ml_dtypes

import concourse.bass as bass
import concourse.bacc as bacc
import concourse.tile as tile
from concourse import mybir
from concourse.bass_utils import run_bass_kernel_spmd

BF16 = mybir.dt.float16  # fp16: same PE rate as bf16, 10-bit mantissa
FP8 = mybir.dt.float8e4
F32 = mybir.dt.float32
AF = mybir.ActivationFunctionType
DR = mybir.MatmulPerfMode.DoubleRow

N_CORES = 8
B, S, E = 4, 8192, 1024
HEADS, HEAD_DIM = 16, 64
M_TOTAL = B * S                # 32768
M_CORE = M_TOTAL // N_CORES    # 4096 rows per core
MS = 512                       # m-stripe width (moving free dim / PSUM bank)
N_STRIPES = M_CORE // MS       # 8
KP = E // 256                  # 4 DoubleRow contraction pair-chunks
NC_ = E // 128                 # 8 feature chunks
W_SCALE = 32.0                 # pre-scale on W1q / W2T before fp8 quantization
OUT_SCALE = W_SCALE * 64.0     # host divides gathered output by this

_BF = np.float16
_F8 = ml_dtypes.float8_e4m3fn

import os
# scheduling experiment knobs (see build_nc)
ACT_DRAINS = int(os.environ.get("K_ACT_DRAINS", "8"))   # first N o-drains on ACT
U8M_POS = os.environ.get("K_U8M_POS", "tail")           # tail | mid
HS_POS = os.environ.get("K_HS_POS", "tail")             # tail | mid
U8M_ENG = os.environ.get("K_U8M_ENG", "dve")            # dve | gp
PSRB = int(os.environ.get("K_PSRB", "2"))               # rb PSUM banks (o gets 5-PSRB)
ILV = int(os.environ.get("K_ILV", "2"))                 # rb per mm2-j-block group


def build_nc() -> bass.Bass:
    nc = bacc.Bacc("TRN2", debug=False)

    # x/w1/w2 are pre-packed on host so every DMA tile is one contiguous
    # block (1-2KB per-partition lines instead of 512B strided runs)
    xt8 = nc.dram_tensor("xt8", [KP, N_STRIPES, 128, 2 * MS], FP8, kind="ExternalInput")
    w18 = nc.dram_tensor("w18", [KP, 128, 2 * E], FP8, kind="ExternalInput")
    w28 = nc.dram_tensor("w28", [KP, 128, 2 * E], FP8, kind="ExternalInput")
    sel8 = nc.dram_tensor("sel8", [128, KP * 2 * HEADS], FP8, kind="ExternalInput")
    selrb = nc.dram_tensor("selrb", [128, NC_ * 2 * 128], FP8, kind="ExternalInput")
    outT = nc.dram_tensor("outT", [E, M_CORE], FP8, kind="ExternalOutput")

    with tile.TileContext(nc) as tc:
        with (
            tc.tile_pool(name="weights", bufs=1) as wpool,
            tc.tile_pool(name="xt", bufs=N_STRIPES) as xpool,
            tc.tile_pool(name="u", bufs=2) as upool,
            tc.tile_pool(name="u8", bufs=2) as u8pool,
            tc.tile_pool(name="at", bufs=2) as apool,
            tc.tile_pool(name="d8", bufs=2) as d8pool,
            tc.tile_pool(name="small", bufs=3) as spool,
            tc.tile_pool(name="ostage", bufs=8) as opool,
            tc.tile_pool(name="ps_q", bufs=3, space="PSUM") as psq,
            tc.tile_pool(name="ps_rb", bufs=PSRB, space="PSUM") as psrb,
            tc.tile_pool(name="ps_o", bufs=5 - PSRB, space="PSUM") as pso,
        ):
            # Stripe-0-critical loads first: w1 pair-chunks interleaved with
            # stripe-0 x pair-chunks, then the tiny sel8.  w2/selrb are
            # deferred until after stripe 1's x loads (first read >=1 stripe
            # in).
            # Warm the PE's HAM clock gate with throwaway matmuls on memset
            # scratch while the first weight/x DMAs are in flight, so the
            # first real matmuls run at 2.4 GHz instead of 1.2.  The memset
            # must be GPSIMD: its ucode memset is up in ~2us, while the
            # DVE's first op waits on a ~5us engine init.
            warm_sb = wpool.tile([128, MS], BF16, name="warm_sb")
            nc.gpsimd.memset(warm_sb[:], 0.0)
            warm_ps = psq.tile([128, MS], F32, tag="q", name="warm_ps")
            for _ in range(8):
                nc.tensor.matmul(
                    warm_ps[:], warm_sb[:, 0:128], warm_sb[:], start=True, stop=True
                )

            # Head loads are split across both HW-DGE rings so the first two
            # DMA-paced iterations finish sooner: w1 + x0[t=0,1] on the SP
            # ring, x0[t=2,3] on the ACT ring (whose ~5us engine init has
            # passed by the time those chunks are needed).
            w1_t = []
            xt0 = []
            for t in range(KP):
                w = wpool.tile([128, 2, E], FP8, tag=f"w1_{t}", name=f"w1t{t}")
                nc.sync.dma_start(w[:], w18[t, :, :])
                w1_t.append(w)
                tx = xpool.tile([128, 2, MS], FP8, tag=f"xt_{t}", name=f"xt0_{t}")
                nc.sync.dma_start(tx[:], xt8[t, 0, :, :])
                xt0.append(tx)

            # Pre-zero the rotating rcp hi/lo pair tiles: only partitions
            # 0:16 are rewritten per stripe, the rest must stay 0 so the
            # DR rb matmul contracts clean zeros (0 * garbage = NaN risk).
            rcphl_bufs = []
            for i in range(3):
                z = spool.tile([128, 2, MS], FP8, tag="rcphl", name=f"rcphl{i}")
                nc.vector.memset(z[:], 0.0)
                rcphl_bufs.append(z)
            sel8_t = wpool.tile([128, KP, 2, HEADS], FP8, name="sel8_t")
            nc.sync.dma_start(
                sel8_t[:],
                sel8[:, :].rearrange("p (t two h) -> p t two h", two=2, h=HEADS),
            )

            selrb_t = wpool.tile([128, NC_, 2, 128], FP8, name="selrb_t")
            w2_k = [
                wpool.tile([128, 2, E], FP8, tag=f"w2_{t}", name=f"w2k{t}")
                for t in range(KP)
            ]

            # ---- software pipeline state ----
            u_hist = {}    # stripe -> [128, NC_, MS] fp16 u strip
            rcp_hist = {}  # stripe -> [128, 2, MS] fp8 hi/lo rcp pair tile
            d8_hist = {}   # stripe -> [128, KP, 2, MS] fp8 centered-attn strip

            for it in range(N_STRIPES + 2):
                cur = it if it < N_STRIPES else None        # mm1/u8/hs stripe
                nrm = it - 1 if 1 <= it <= N_STRIPES else None  # rb/at/d8
                prj = it - 2 if it >= 2 else None           # mm2 stripe

                # ---- x DMA for the current stripe ----
                if cur is not None and cur > 0:
                    xt_k = []
                    for t in range(KP):
                        tx = xpool.tile(
                            [128, 2, MS], FP8, tag=f"xt_{t}", name=f"xt{cur}_{t}"
                        )
                        nc.sync.dma_start(tx[:], xt8[t, cur, :, :])
                        xt_k.append(tx)
                elif cur == 0:
                    xt_k = xt0
                if it == 1:
                    nc.sync.dma_start(
                        selrb_t[:],
                        selrb[:, :].rearrange(
                            "p (c two q) -> p c two q", two=2, q=128
                        ),
                    )
                    for t in range(KP):
                        nc.sync.dma_start(w2_k[t][:], w28[t, :, :])

                # ---- mm1(cur): q-projection, 32-MM fp8-DR block + exp ----
                if cur is not None:
                    u_strip = upool.tile([128, NC_, MS], BF16, tag="u", name="u_strip")
                    for ci in range(NC_):
                        q_ps = psq.tile([128, MS], F32, tag="q", name="q_ps")
                        for t in range(KP):
                            nc.tensor.matmul(
                                q_ps[:],
                                w1_t[t][:, :, ci * 128:(ci + 1) * 128],
                                xt_k[t][:],
                                start=(t == 0),
                                stop=(t == KP - 1),
                                perf_mode=DR,
                            )
                        nc.scalar.activation(
                            u_strip[:, ci, :], q_ps[:], AF.Exp,
                            scale=1.0 / (4.0 * W_SCALE),
                        )
                    u_hist[cur] = u_strip

                # ---- rb/at(nrm) interleaved with mm2(prj) ----
                # DVE FIFO must run [at0..at6][u8m][at7][dj6][dj7][recip]
                # [lo8]: the at-muls pace the 2-bank rb PSUM rotation, u8m
                # slots in before the head-sum needs it, and only the last
                # two o-drains ride the DVE (the rest go to ACT).
                if nrm is not None:
                    at_strip = apool.tile(
                        [128, NC_, MS], BF16, tag="at", name="at_strip"
                    )
                u8_strip = None

                def emit_rb(ci):
                    rb_ps = psrb.tile([128, MS], F32, tag="rb", name="rb_ps")
                    nc.tensor.matmul(
                        rb_ps[:], selrb_t[:, ci, :, :], rcp_hist[nrm][:],
                        start=True, stop=True, perf_mode=DR,
                    )
                    nc.vector.tensor_mul(
                        at_strip[:, ci, :], u_hist[nrm][:, ci, :], rb_ps[:]
                    )

                def emit_j(j):
                    o_ps = pso.tile([128, MS], F32, tag="o", name="o_ps")
                    for t in range(KP):
                        nc.tensor.matmul(
                            o_ps[:],
                            w2_k[t][:, :, j * 128:(j + 1) * 128],
                            d8_hist[prj][:, t, :, :],
                            start=(t == 0),
                            stop=(t == KP - 1),
                            perf_mode=DR,
                        )
                    o_t = opool.tile([128, MS], FP8, tag="ost", name="o_t")
                    if j < ACT_DRAINS:
                        nc.scalar.copy(o_t[:], o_ps[:])
                    else:
                        nc.vector.tensor_scalar_mul(o_t[:], o_ps[:], 1.0)
                    nc.sync.dma_start(
                        outT[j * 128:(j + 1) * 128, prj * MS:(prj + 1) * MS], o_t[:]
                    )

                def emit_u8m():
                    # one whole-strip fp8 copy for the DR head-sum; the
                    # [t, i] pair-interleave order IS ci order
                    nonlocal u8_strip
                    u8_strip = u8pool.tile(
                        [128, KP, 2, MS], FP8, tag="u8", name="u8_strip"
                    )
                    eng = nc.gpsimd if U8M_ENG == "gp" else nc.vector
                    eng.tensor_scalar_mul(
                        u8_strip[:, :, :, :].rearrange("p t i m -> p (t i m)"),
                        u_strip[:, :, :].rearrange("p c m -> p (c m)"),
                        1.0,
                    )

                def emit_hs():
                    # head sums (4-MM fp8-DR block) + hi/lo fp8 reciprocal
                    s_ps = psrb.tile([128, MS], F32, tag="rb", name="s_ps")
                    for t in range(KP):
                        nc.tensor.matmul(
                            s_ps[0:HEADS, :],
                            sel8_t[:, t, :, :],
                            u8_strip[:, t, :, :],
                            start=(t == 0),
                            stop=(t == KP - 1),
                            perf_mode=DR,
                        )
                    rcp32 = spool.tile([HEADS, MS], F32, tag="rcp32", name="rcp32")
                    nc.vector.reciprocal_approx_fast(rcp32[:], s_ps[0:HEADS, :])
                    rcphl = rcphl_bufs[cur % 3]
                    nc.scalar.copy(rcphl[0:HEADS, 0, :], rcp32[:])
                    nc.vector.tensor_sub(
                        rcphl[0:HEADS, 1, :], rcp32[:], rcphl[0:HEADS, 0, :]
                    )
                    rcp_hist[cur] = rcphl

                hs_mid = cur is not None and HS_POS == "mid"
                u8_mid = U8M_POS == "mid"
                if nrm is not None and prj is not None and cur is None:
                    # epilogue iteration: no mm1 block runs ahead of the rb
                    # matmuls, so lead with mm2 j-blocks to cover the
                    # recip->hi8->lo8 chain of the final stripe
                    emit_j(0)
                    emit_j(1)
                    for g in range(3):
                        emit_rb(2 * g)
                        emit_rb(2 * g + 1)
                        emit_j(g + 2)
                    emit_rb(6)
                    emit_rb(7)
                    for j in range(5, NC_):
                        emit_j(j)
                elif nrm is not None and prj is not None and ILV == 1:
                    # 1:1 interleave: rb matmuls 1065ns apart so the 2-bank
                    # rb rotation never waits on a DVE at-mul drain; u8m is
                    # split in half and slotted into the DVE FIFO where the
                    # at-mul stream has slack
                    def emit_u8h(i):
                        nonlocal u8_strip
                        if i == 0:
                            u8_strip = u8pool.tile(
                                [128, KP, 2, MS], FP8, tag="u8", name="u8_strip"
                            )
                        nc.vector.tensor_scalar_mul(
                            u8_strip[:, 2 * i:2 * i + 2, :, :].rearrange(
                                "p t i m -> p (t i m)"
                            ),
                            u_strip[:, 4 * i:4 * i + 4, :].rearrange(
                                "p c m -> p (c m)"
                            ),
                            1.0,
                        )
                    for i in range(NC_):
                        emit_rb(i)
                        if cur is not None and i == 1:
                            emit_u8h(0)
                        if cur is not None and i == 4:
                            emit_u8h(1)
                        emit_j(i)
                elif nrm is not None and prj is not None:
                    if cur is not None and (u8_mid or hs_mid):
                        emit_u8m()
                    for g in range(3):
                        emit_rb(2 * g)
                        emit_rb(2 * g + 1)
                        emit_j(g)
                        if g == 1 and hs_mid:
                            emit_hs()
                    emit_rb(6)
                    if cur is not None and not (u8_mid or hs_mid):
                        emit_u8m()
                    emit_rb(7)
                    emit_j(3)
                    for j in range(4, NC_):
                        emit_j(j)
                elif nrm is not None:
                    if cur is not None and (u8_mid or hs_mid):
                        emit_u8m()
                    for ci in range(NC_):
                        emit_rb(ci)
                        if ci == 5 and hs_mid:
                            emit_hs()
                        if ci == 6 and cur is not None and not (u8_mid or hs_mid):
                            emit_u8m()
                elif prj is not None:
                    if cur is not None:
                        emit_u8m()
                        if hs_mid:
                            emit_hs()
                    for j in range(NC_):
                        emit_j(j)
                elif cur is not None:
                    emit_u8m()
                    if hs_mid:
                        emit_hs()

                # ---- d8(nrm): fp8(64*attn - 1) in one DVE pass ----
                if nrm is not None:
                    d8_strip = d8pool.tile(
                        [128, KP, 2, MS], FP8, tag="d8", name="d8_strip"
                    )
                    nc.vector.tensor_scalar_add(
                        d8_strip[:, :, :, :].rearrange("p t i m -> p (t i m)"),
                        at_strip[:, :, :].rearrange("p c m -> p (c m)"),
                        -1.0,
                    )
                    d8_hist[nrm] = d8_strip
                    del u_hist[nrm]
                    del rcp_hist[nrm]
                if prj is not None:
                    del d8_hist[prj]

                # ---- head sums + reciprocal (unless already emitted mid) ----
                if cur is not None and not hs_mid:
                    emit_hs()
    nc.compile()
    return nc


_NC_CACHE = None
LAST_RESULT = None


def _ensure_ntff_hook():
    """bass_utils' axon trace path needs antenv.axon_hooks, which this
    container's antenv lacks. Provide it + register the ctypes NTFF hook."""
    import types

    try:
        from antenv.axon_hooks import get_axon_ntff_profile_hook  # noqa: F401
        return True
    except ImportError:
        pass
    try:
        import antenv
        from trn_agent_boot.trn_boot import _ntff_profile_via_ctypes

        m = types.ModuleType("antenv.axon_hooks")
        state = {"hook": None}
        m.set_axon_ntff_profile_hook = lambda h: state.__setitem__("hook", h)
        m.get_axon_ntff_profile_hook = lambda: state["hook"]
        sys.modules["antenv.axon_hooks"] = m
        antenv.axon_hooks = m
        m.set_axon_ntff_profile_hook(
            _ntff_profile_via_ctypes("/opt/axon/libaxon_pjrt.so")
        )
        return True
    except Exception as e:  # pragma: no cover
        print(f"ntff hook injection failed: {e}")
        return False


def _selectors():
    # head index of global feature n is n // 64; pair-chunk t group i covers
    # chunk ci = 2t+i, i.e. heads 2ci (partitions 0..63) and 2ci+1 (64..127).
    # Entries are 1/64 (exact in fp8) so the head-sum PSUM holds s/64.
    sel8 = np.zeros((128, KP, 2, HEADS), np.float32)
    for t in range(KP):
        for i in range(2):
            ci = 2 * t + i
            sel8[:64, t, i, 2 * ci] = 1.0 / 64.0
            sel8[64:, t, i, 2 * ci + 1] = 1.0 / 64.0
    # rb broadcast selector: chunk ci output partition p needs head
    # 2ci + (p >= 64); contraction runs over the hi/lo pair (i = 0, 1) of
    # partition h of the rcphl tile, both with weight 1 -> rb = hi8 + lo8.
    selrb = np.zeros((128, NC_, 2, 128), np.float32)
    for ci in range(NC_):
        for i in range(2):
            selrb[2 * ci, ci, i, :64] = 1.0
            selrb[2 * ci + 1, ci, i, 64:] = 1.0
    return (
        np.ascontiguousarray(sel8.reshape(128, KP * 2 * HEADS)).astype(_F8),
        np.ascontiguousarray(selrb.reshape(128, NC_ * 2 * 128)).astype(_F8),
    )


def _pack_pairs(wT):
    """[k, n] -> [KP, 128, 2*n] with k = 256t + 128i + p pair interleave."""
    n = wT.shape[1]
    return np.ascontiguousarray(
        wT.reshape(KP, 2, 128, n).transpose(0, 2, 1, 3).reshape(KP, 128, 2 * n)
    )


def kernel(x, W1, W2, heads, trace=False):
    global _NC_CACHE, LAST_RESULT
    x = np.asarray(x, dtype=np.float32)
    W1 = np.asarray(W1, dtype=np.float32)
    W2 = np.asarray(W2, dtype=np.float32)

    X = x.reshape(M_TOTAL, E)
    X8T = np.ascontiguousarray(X.T).astype(_F8)           # [E, M_TOTAL]
    w18p = _pack_pairs((W1[:E, :] * W_SCALE).T.astype(_F8))   # q-proj weights
    w28p = _pack_pairs((W2.T * W_SCALE).astype(_F8))          # [n, j] = W2[j, n]
    sel8, selrb = _selectors()
    # constant part of the output: sum_n W2T[n,j] * (1/64)
    Kj = W2.astype(np.float64).sum(axis=1) / 64.0         # [E], index j

    in_maps = []
    for c in range(N_CORES):
        xt_c = X8T[:, c * M_CORE:(c + 1) * M_CORE]
        xt_p = np.ascontiguousarray(
            xt_c.reshape(KP, 2, 128, N_STRIPES, MS)
            .transpose(0, 3, 2, 1, 4)
            .reshape(KP, N_STRIPES, 128, 2 * MS)
        )
        in_maps.append(
            {"xt8": xt_p, "w18": w18p, "w28": w28p, "sel8": sel8, "selrb": selrb}
        )

    if _NC_CACHE is None:
        _NC_CACHE = build_nc()

    if trace:
        trace = _ensure_ntff_hook()

    res = run_bass_kernel_spmd(_NC_CACHE, in_maps, list(range(N_CORES)), trace=trace)
    LAST_RESULT = res

    OT = np.concatenate(
        [np.asarray(res.results[c]["outT"]).astype(np.float32) for c in range(N_CORES)],
        axis=1,
    )
    out = OT.T * np.float32(1.0 / OUT_SCALE) + Kj.astype(np.float32)[None, :]
    return np.ascontiguousarray(out).reshape(B, S, E)
